# revision 41
# baseline (speedup 1.0000x reference)
"""Trainium2 Bass kernel: frame-block-causal multi-head attention with LayerNorm.

Full module: LayerNorm(x) -> QKV proj -> 16-head block-causal attention
(8 frames x 256 patches) -> output projection.

Sharding: 8 cores = batch(2) x head-groups(4 heads each).  Each core gets its
batch's x and the weight column/row slices for its 4 heads, computes a partial
output [2048, 1024]; host sums the 4 partials per batch.  No collectives.

v2 schedule: fully pipelined single-pass emission.
 - x arrives bf16; LN per 128-seq tile on DVE; xhat transposed to SBUF
   feature-major layout via xbar DMA transpose (SBUF->SBUF, no DRAM bounce).
 - QKV projection per 512-seq block, interleaved with attention panels so the
   PE never drains: block b QKV -> attention panels (frames 2b,2b+1 for all 4
   heads) -> block b+1 QKV -> ...
 - Attention per panel (512 queries, one head): S matmuls (K=64) grouped into
   [128,1024] PSUM tiles, one Exp per group on ACT, AV accumulation (M=65,
   ones column produces softmax denominators), denominator broadcast via K=1
   matmul, normalize on DVE.  One-panel software pipeline lookahead keeps
   ACT exp off the PE critical path.
 - Output projection per block as soon as all 4 heads' panels normalize.

All matmuls bf16 with fp32 PSUM.  LN statistics fp32 (bf16 inputs).
"""

import numpy as np
import ml_dtypes

import concourse.bass as bass
import concourse.mybir as mybir
import concourse.tile as tile
from concourse import bacc, bass_utils

# ---- problem constants (hardcoded; kernel.py must be self-contained) ----
DIM = 1024
HEADS = 16
DIM_HEAD = 64
NUM_FRAMES = 8
PATCHES_PER_FRAME = 256
BATCH = 2
SEQ = NUM_FRAMES * PATCHES_PER_FRAME  # 2048
EPS = 1e-5
SCALE = DIM_HEAD ** -0.5  # 0.125

N_CORES = 8
HG = 4            # heads per core
NT = SEQ // 128   # 16 seq tiles of 128
NCK = DIM // 128  # 8 contraction chunks
NB = 4            # seq blocks of 512

dt = mybir.dt
AF = mybir.ActivationFunctionType
ALU = mybir.AluOpType


def build_program():
    nc = bacc.Bacc("TRN2", target_bir_lowering=False, debug=False)
    f32, bf16 = dt.float32, dt.bfloat16

    x_d = nc.dram_tensor("x", [SEQ, DIM], bf16, kind="ExternalInput")
    wq_d = nc.dram_tensor("wq", [DIM, HG * DIM_HEAD], bf16, kind="ExternalInput")
    wk_d = nc.dram_tensor("wk", [DIM, HG * DIM_HEAD], bf16, kind="ExternalInput")
    wv_d = nc.dram_tensor("wv", [DIM, HG * DIM_HEAD], bf16, kind="ExternalInput")
    wo_d = nc.dram_tensor("wo", [HG * DIM_HEAD, DIM], bf16, kind="ExternalInput")
    bqk_d = nc.dram_tensor("bqk", [128, 4], f32, kind="ExternalInput")
    out_d = nc.dram_tensor("out", [SEQ, DIM], f32, kind="ExternalOutput")

    with tile.TileContext(nc) as tc:
        with tc.tile_pool(name="persist", bufs=1) as persist:
            _build_body(tc, nc, persist, x_d, wq_d, wk_d, wv_d, wo_d, bqk_d, out_d)
    nc.compile()
    return nc


def _build_body(tc, nc, persist, x_d, wq_d, wk_d, wv_d, wo_d, bqk_d, out_d):
    f32, bf16 = dt.float32, dt.bfloat16

    # persistent SBUF tensors
    xhatT = persist.tile([128, NCK, SEQ], bf16)          # x-hat transposed, chunked
    wq_sb = persist.tile([128, NCK, HG * DIM_HEAD], bf16)
    wk_sb = persist.tile([128, NCK, HG * DIM_HEAD], bf16)
    wv_sb = persist.tile([128, NCK, HG * DIM_HEAD], bf16)
    wo_sb = persist.tile([128, 2, 2, 512], bf16)         # [pair-row, jc, half, n]
    bqk_sb = persist.tile([128, 4], f32)
    qt_p = persist.tile([128, 2, SEQ], bf16)             # Q^T head pairs
    kt_p = persist.tile([128, 2, SEQ], bf16)             # K^T head pairs
    v1 = persist.tile([128, NT, HG, DIM_HEAD + 1], bf16)  # V natural + ones col
    at_p = persist.tile([128, 2, SEQ], bf16)             # attn_out^T head pairs
    epst = persist.tile([128, 1], f32)
    ones64 = persist.tile([128, 64], bf16)

    nc.vector.memset(epst[:], EPS)
    nc.vector.memset(ones64[:], 1.0)
    nc.vector.memset(v1[:], 1.0)

    nc.sync.dma_start(bqk_sb[:], bqk_d.ap())
    nc.sync.dma_start(wq_sb[:], wq_d.ap().rearrange("(c p) j -> p c j", p=128))
    nc.sync.dma_start(wk_sb[:], wk_d.ap().rearrange("(c p) j -> p c j", p=128))
    nc.sync.dma_start(wv_sb[:], wv_d.ap().rearrange("(c p) j -> p c j", p=128))
    nc.sync.dma_start(
        wo_sb[:], wo_d.ap().rearrange("(jc p) (hf n) -> p jc hf n", p=128, n=512)
    )

    with (
        tc.tile_pool(name="xin", bufs=16) as xin,
        tc.tile_pool(name="stat", bufs=3) as statp,
        tc.tile_pool(name="xh", bufs=3) as xhp,
        tc.tile_pool(name="exs", bufs=18) as expool,
        tc.tile_pool(name="nrm", bufs=3) as nrm,
        tc.tile_pool(name="osb", bufs=2) as osb,
        tc.tile_pool(name="psqk", bufs=2, space="PSUM") as psQK,
        tc.tile_pool(name="pss", bufs=2, space="PSUM") as psS,
        tc.tile_pool(name="psav", bufs=2, space="PSUM") as psAV,
    ):
        # all x tile loads dispatched upfront: no waits, so the sync queue
        # never head-of-line blocks a later x load behind a transpose DMA
        xts = []
        for t in range(NT):
            xt = xin.tile([128, DIM], bf16, tag="x", name=f"x_{t}")
            nc.sync.dma_start(xt[:], x_d.ap()[t * 128:(t + 1) * 128, :])
            xts.append(xt)

        # ---- LayerNorm + transpose, one 128-seq tile ----
        def emit_ln_tile(t):
            xt = xts[t]
            bn = statp.tile([128, 12], f32, tag="bn", name=f"bn_{t}")
            nc.vector.bn_stats(bn[:, 0:6], xt[:, 0:512])
            nc.vector.bn_stats(bn[:, 6:12], xt[:, 512:1024])
            mv = statp.tile([128, 2], f32, tag="mv", name=f"mv_{t}")
            nc.vector.bn_aggr(mv[:], bn[:])
            std = statp.tile([128, 1], f32, tag="std", name=f"std_{t}")
            nc.scalar.activation(std[:], mv[:, 1:2], AF.Sqrt, bias=epst[:], scale=1.0)
            rstd = statp.tile([128, 1], f32, tag="rstd", name=f"rstd_{t}")
            nc.vector.reciprocal(rstd[:], std[:])
            nbias = statp.tile([128, 1], f32, tag="nb", name=f"nb_{t}")
            nc.vector.scalar_tensor_tensor(
                nbias[:], mv[:, 0:1], -1.0, rstd[:], ALU.mult, ALU.mult
            )
            xh = xhp.tile([128, DIM], bf16, tag="xh", name=f"xh_{t}")
            nc.vector.tensor_scalar(xh[:], xt[:], rstd[:], nbias[:], ALU.mult, ALU.add)
            # xbar transpose SBUF->SBUF: xhatT[d%128, d//128, t*128+s] = xh[s, d]
            nc.sync.dma_start_transpose(xhatT[:, :, t * 128:(t + 1) * 128], xh[:])

        # ---- QKV projection for one 512-seq block ----
        def emit_qkv_block(b):
            s0 = b * 512
            # V first: per-tile dependency lets the PE start earliest
            for t in range(4 * b, 4 * b + 4):
                vps = psQK.tile([128, 512], f32, tag="qk", name=f"v_{t}")
                for ci in range(NCK):
                    nc.tensor.matmul(
                        vps[:, 0:256], xhatT[:, ci, t * 128:(t + 1) * 128],
                        wv_sb[:, ci, :],
                        start=(ci == 0), stop=(ci == NCK - 1),
                    )
                nc.vector.tensor_copy(
                    v1[:, t, :, 0:DIM_HEAD],
                    vps[:, 0:256].rearrange("p (h d) -> p h d", h=HG),
                )
            for jc in range(2):
                qps = psQK.tile([128, 512], f32, tag="qk", name=f"q_{b}_{jc}")
                for ci in range(NCK):
                    nc.tensor.matmul(
                        qps[:], wq_sb[:, ci, jc * 128:(jc + 1) * 128],
                        xhatT[:, ci, s0:s0 + 512],
                        start=(ci == 0), stop=(ci == NCK - 1),
                    )
                nc.scalar.activation(
                    qt_p[:, jc, s0:s0 + 512], qps[:], AF.Identity,
                    bias=bqk_sb[:, jc:jc + 1], scale=1.0,
                )
                kps = psQK.tile([128, 512], f32, tag="qk", name=f"k_{b}_{jc}")
                for ci in range(NCK):
                    nc.tensor.matmul(
                        kps[:], wk_sb[:, ci, jc * 128:(jc + 1) * 128],
                        xhatT[:, ci, s0:s0 + 512],
                        start=(ci == 0), stop=(ci == NCK - 1),
                    )
                nc.scalar.activation(
                    kt_p[:, jc, s0:s0 + 512], kps[:], AF.Identity,
                    bias=bqk_sb[:, 2 + jc:3 + jc], scale=1.0,
                )

        # ---- attention panel: frames 2b,2b+1 (512 queries) for head h ----
        def panel_slices(b):
            # (kb, qoff, qlen): key block kb attends queries [b*512+qoff, b*512+512)
            out = []
            for kb in range(4 * b + 4):
                qoff = 256 if kb >= 4 * b + 2 else 0
                out.append((kb, qoff, 512 - qoff))
            return out

        def emit_panel_S(b, h):
            jc, lo = h // 2, (h % 2) * 64
            q0 = b * 512
            groups, cur, cw = [], [], 0
            for s in panel_slices(b):
                if cw + s[2] > 1024:
                    groups.append(cur)
                    cur, cw = [], 0
                cur.append(s)
                cw += s[2]
            if cur:
                groups.append(cur)
            ex_list = []
            for gi, g in enumerate(groups):
                w = sum(s[2] for s in g)
                sps = psS.tile([128, 1024], f32, tag="s", name=f"s_{b}_{h}_{gi}")
                off = 0
                for (kb, qoff, qlen) in g:
                    nc.tensor.matmul(
                        sps[:, off:off + qlen],
                        kt_p[lo:lo + 64, jc, kb * 128:(kb + 1) * 128],
                        qt_p[lo:lo + 64, jc, q0 + qoff:q0 + 512],
                        start=True, stop=True,
                    )
                    off += qlen
                ex = expool.tile([128, 1024], bf16, tag="ex", name=f"ex_{b}_{h}_{gi}")
                nc.scalar.activation(
                    ex[:, 0:w], sps[:, 0:w], AF.Exp, bias=0.0, scale=SCALE
                )
                ex_list.append((g, ex))
            return ex_list

        def emit_panel_AV_norm(b, h, ex_list):
            jc, lo = h // 2, (h % 2) * 64
            n = 4 * b + 4
            av = psAV.tile([65, 512], f32, tag="av", name=f"av_{b}_{h}")
            i = 0
            for (g, ex) in ex_list:
                off = 0
                for (kb, qoff, qlen) in g:
                    nc.tensor.matmul(
                        av[:, qoff:512],
                        v1[:, kb, h, :],
                        ex[:, off:off + qlen],
                        start=(i == 0), stop=(i == n - 1),
                    )
                    off += qlen
                    i += 1
            # normalize: denom row 64 -> bf16 row -> K=1 broadcast -> recip -> mult
            scr = nrm.tile([65, 512], bf16, tag="scr", name=f"scr_{b}_{h}")
            nc.vector.tensor_copy(scr[64:65, :], av[64:65, :])
            bps = psS.tile([128, 1024], f32, tag="s", name=f"bc_{b}_{h}")
            nc.tensor.matmul(
                bps[0:64, 0:512], ones64[64:65, :], scr[64:65, :],
                start=True, stop=True,
            )
            rec = nrm.tile([64, 512], f32, tag="rec", name=f"rec_{b}_{h}")
            nc.vector.reciprocal_approx_fast(rec[:], bps[0:64, 0:512])
            nc.vector.tensor_tensor(
                at_p[lo:lo + 64, jc, b * 512:(b + 1) * 512],
                av[0:64, :], rec[:], ALU.mult,
            )

        # ---- output projection for one 128-seq tile ----
        def emit_outproj(t):
            ot = osb.tile([128, DIM], f32, tag="o", name=f"o_{t}")
            for hf in range(2):
                ops = psQK.tile([128, 512], f32, tag="qk", name=f"d_{t}_{hf}")
                for jc in range(2):
                    nc.tensor.matmul(
                        ops[:],
                        at_p[:, jc, t * 128:(t + 1) * 128],
                        wo_sb[:, jc, hf, :],
                        start=(jc == 0), stop=(jc == 1),
                    )
                if hf == 0:
                    nc.scalar.copy(ot[:, 0:512], ops[:])
                else:
                    nc.vector.tensor_copy(ot[:, 512:1024], ops[:])
            nc.sync.dma_start(out_d.ap()[t * 128:(t + 1) * 128, :], ot[:])

        # ---- main emission: blocks pipelined with 1-panel lookahead ----
        for t in range(4):
            emit_ln_tile(t)
        pend = None
        for b in range(NB):
            emit_qkv_block(b)
            if b + 1 < NB:
                for t in range(4 * (b + 1), 4 * (b + 1) + 4):
                    emit_ln_tile(t)
            for h in range(HG):
                exl = emit_panel_S(b, h)
                if pend is not None:
                    emit_panel_AV_norm(*pend)
                    if pend[1] == HG - 1:
                        for t in range(4 * pend[0], 4 * pend[0] + 4):
                            emit_outproj(t)
                pend = (b, h, exl)
        emit_panel_AV_norm(*pend)
        for t in range(4 * (NB - 1), 4 * (NB - 1) + 4):
            emit_outproj(t)


_CACHED_NC = None


def _get_nc():
    global _CACHED_NC
    if _CACHED_NC is None:
        _CACHED_NC = build_program()
    return _CACHED_NC


def shard_inputs(x, g, b, w_qkv, w_out, b_out):
    """Host-side prep: fold LN gain into w_qkv, slice per core."""
    bf16 = ml_dtypes.bfloat16
    g_f = np.asarray(g, np.float32).reshape(-1)
    b_f = np.asarray(b, np.float32).reshape(-1)
    w_qkv = np.asarray(w_qkv, np.float32)
    w_out = np.asarray(w_out, np.float32)
    wg = w_qkv * g_f[:, None]            # fold gain
    bqkv = b_f @ w_qkv                   # [3072] qkv bias from LN beta

    in_maps = []
    host_bias = np.zeros((BATCH, DIM), np.float32)
    x_bf = [
        np.ascontiguousarray(np.asarray(x, np.float32)[bb].astype(bf16))
        for bb in range(BATCH)
    ]
    for core in range(N_CORES):
        bb = core // 4
        hg = core % 4
        h0 = hg * HG
        c0 = h0 * DIM_HEAD
        sl = slice(c0, c0 + HG * DIM_HEAD)
        wq_c = wg[:, sl].astype(bf16)
        wk_c = wg[:, DIM + c0: DIM + c0 + HG * DIM_HEAD].astype(bf16)
        wv_c = wg[:, 2 * DIM + c0: 2 * DIM + c0 + HG * DIM_HEAD].astype(bf16)
        wo_c = w_out[sl, :].astype(bf16)
        bqk_c = np.zeros((128, 4), np.float32)
        bqk_c[:, 0] = bqkv[c0: c0 + 128]
        bqk_c[:, 1] = bqkv[c0 + 128: c0 + 256]
        bqk_c[:, 2] = bqkv[DIM + c0: DIM + c0 + 128]
        bqk_c[:, 3] = bqkv[DIM + c0 + 128: DIM + c0 + 256]
        # v-bias folds exactly into a constant output bias (attn rows sum to 1)
        bv_c = bqkv[2 * DIM + c0: 2 * DIM + c0 + HG * DIM_HEAD]
        host_bias[bb] += bv_c @ w_out[sl, :]
        in_maps.append({
            "x": x_bf[bb],
            "wq": np.ascontiguousarray(wq_c),
            "wk": np.ascontiguousarray(wk_c),
            "wv": np.ascontiguousarray(wv_c),
            "wo": np.ascontiguousarray(wo_c),
            "bqk": bqk_c,
        })
    return in_maps, host_bias


def kernel(x, g, b, w_qkv, w_out, b_out, _results_hook=None):
    nc = _get_nc()
    in_maps, host_bias = shard_inputs(x, g, b, w_qkv, w_out, b_out)
    res = bass_utils.run_bass_kernel_spmd(nc, in_maps, core_ids=list(range(N_CORES)))
    if _results_hook is not None:
        _results_hook(res)
    out = np.zeros((BATCH, SEQ, DIM), np.float32)
    for core in range(N_CORES):
        out[core // 4] += res.results[core]["out"]
    out += host_bias[:, None, :]
    out += np.asarray(b_out, np.float32)[None, None, :]
    return out
